# revision 1
# baseline (speedup 1.0000x reference)
"""HeadFusionAttention Trainium2 kernel (8 NeuronCores, data-parallel over B).

Reference computation (per batch b):
    head_x = 0
    for i in 0..3:                                  # sequential group chain
        cur   = x[:, 256*i:256*(i+1)] + head_x      # [N, 256]
        qkv   = cur @ qkv_w[i].T                    # [N, 768] -> q,k,v [N,256]
        S     = (q @ k.T) * SCALE                   # [N, N]
        P     = softmax(S, axis=-1)
        head_x = P @ v                              # [N, 256]
        y[:, 256*i:256*(i+1)] = head_x
    out = y @ proj_w.T + proj_b                     # [N, 1024]

Sharding: B=16 batches split 2 per core across 8 cores; weights replicated;
zero collectives. All activations are kept transposed on-chip ([feature, n]
layout) so every matmul consumes operands in natural [K, M]/[K, N] form and
no on-chip transposes are needed:
  - qkv^T = wT.T-matmul(cur^T)            (q^T, k^T in [d, n])
  - v is produced directly in [n, d] via swapping matmul operands
  - S^T   = k^T-matmul(q^T)  ([k_n, q_n]); softmax runs along partitions:
    exp via ACT (scale folded in, no max subtraction -- scores are O(1)),
    denominators via ones-vector matmul, broadcast via stride-0-partition DMA
  - head_x^T = v-matmul(P^T), normalized by reciprocal denominators
  - out^T accumulates per-group projection partials (bias folded into the
    first partial); host transposes the [e, n] result back.
"""

import numpy as np

B, N, DIM = 16, 1024, 1024
G = 4
G_DIM = 256
SCALE = 128 ** -0.5
N_CORES = 8
B_PER = B // N_CORES  # 2

P = 128          # SBUF partitions
FH = 512         # free-dim half (psum bank: 512 fp32)
USE_F32R = True  # fp32r matmuls: full-rate single pass (vs 4x slower fp32)


def build_nc(use_f32r=USE_F32R):
    from contextlib import ExitStack

    import concourse.mybir as mybir
    import concourse.tile as tile
    from concourse import bacc

    f32 = mybir.dt.float32
    # float32r: same 4-byte layout as fp32 but single-pass full-rate matmul.
    # The BIR verifier requires every matmul operand's producer to emit
    # float32r, so all matmul-feeding tiles/params are typed float32r.
    mdt = mybir.dt.float32r if use_f32r else f32

    # Bacc (vs plain Bass) runs the wait-splitting passes walrus requires
    nc = bacc.Bacc()

    xT = nc.declare_dram_parameter("xT", [B_PER, DIM, N], mdt, isOutput=False)
    wqkvT = nc.declare_dram_parameter("wqkvT", [G, G_DIM, 3 * G_DIM], mdt, isOutput=False)
    pwT = nc.declare_dram_parameter("pwT", [DIM, DIM], mdt, isOutput=False)
    pb = nc.declare_dram_parameter("pb", [P, DIM // P], f32, isOutput=False)
    outT = nc.declare_dram_parameter("outT", [B_PER, DIM, N], f32, isOutput=True)

    Exp = mybir.ActivationFunctionType.Exp
    Ident = mybir.ActivationFunctionType.Identity

    with tile.TileContext(nc) as tc, ExitStack() as ctx:
        consts = ctx.enter_context(tc.tile_pool(name="consts", bufs=1))
        pw_pool = ctx.enter_context(tc.tile_pool(name="pw_pool", bufs=2))
        acc_pool = ctx.enter_context(tc.tile_pool(name="acc_pool", bufs=1))
        cur_pool = ctx.enter_context(tc.tile_pool(name="cur_pool", bufs=3))
        qk_pool = ctx.enter_context(tc.tile_pool(name="qk_pool", bufs=1))
        v_pool = ctx.enter_context(tc.tile_pool(name="v_pool", bufs=2))
        pt_pool = ctx.enter_context(tc.tile_pool(name="pt_pool", bufs=4))
        hx_pool = ctx.enter_context(tc.tile_pool(name="hx_pool", bufs=3))
        sm_pool = ctx.enter_context(tc.tile_pool(name="sm_pool", bufs=2))

        ps_mm = ctx.enter_context(tc.tile_pool(name="ps_mm", bufs=2, space="PSUM"))
        ps_s = ctx.enter_context(tc.tile_pool(name="ps_s", bufs=2, space="PSUM"))
        ps_pv = ctx.enter_context(tc.tile_pool(name="ps_pv", bufs=3, space="PSUM"))
        ps_den = ctx.enter_context(tc.tile_pool(name="ps_den", bufs=1, space="PSUM"))

        # ---- constants ----
        # qkv weights, transposed: [d partition, group, d-subtile, e]
        wq_sb = consts.tile([P, G, 2, 3 * G_DIM], mdt)
        nc.sync.dma_start(
            out=wq_sb,
            in_=wqkvT.rearrange("g (ds p) e -> p g ds e", p=P),
        )
        pb_sb = consts.tile([P, DIM // P], f32)
        nc.sync.dma_start(out=pb_sb, in_=pb[:, :])
        # memset can't write float32r directly; stage via f32 + copy
        ones_f32 = consts.tile([P, P], f32)
        nc.vector.memset(ones_f32, 1.0)
        ones_col = consts.tile([P, 1], mdt)
        nc.vector.tensor_copy(ones_col, ones_f32[:, 0:1])
        ones_sq = consts.tile([P, P], mdt)
        nc.vector.tensor_copy(ones_sq, ones_f32)
        zero_f32 = consts.tile([P, FH], f32)
        nc.vector.memset(zero_f32, 0.0)
        # reciprocal staging tiles: row 0 carries data, rows 1.. stay zero so
        # an all-ones matmul broadcasts row 0 across all partitions
        rec_pads = []
        rec_rows = []
        for hh in range(2):
            rec_pad = consts.tile([P, FH], mdt, name=f"rec_pad{hh}")
            nc.vector.tensor_copy(rec_pad, zero_f32)
            rec_pads.append(rec_pad)
            rec_row = consts.tile([1, FH], f32, name=f"rec_row{hh}")
            rec_rows.append(rec_row)

        for b in range(B_PER):
            out_acc = acc_pool.tile([P, DIM // P, N], f32)

            # first group's input: x^T rows 0:256
            cur = cur_pool.tile([P, 2, N], mdt, tag="cur")
            nc.sync.dma_start(
                out=cur, in_=xT[b, 0:G_DIM].rearrange("(ds p) n -> p ds n", p=P)
            )

            prev_hx = None  # (hx_tile, group_idx) pending projection partial
            for i in range(G):
                # projection weight slice for this group, prefetched
                pw_s = pw_pool.tile([P, 2, DIM], mdt)
                nc.sync.dma_start(
                    out=pw_s,
                    in_=pwT[G_DIM * i : G_DIM * (i + 1)].rearrange(
                        "(ds p) e -> p ds e", p=P
                    ),
                )

                # ---- A: q^T, k^T [e-chunk, n] = w_qk.T-matmul(cur^T) ----
                qkT = qk_pool.tile([P, 4, N], mdt)
                for ec in range(4):
                    for h in range(2):
                        ps = ps_mm.tile([P, FH], f32, tag="ps_mm")
                        for ds in range(2):
                            nc.tensor.matmul(
                                ps,
                                (wq_sb[:, i, ds, P * ec : P * (ec + 1)]),
                                (cur[:, ds, FH * h : FH * (h + 1)]),
                                start=(ds == 0),
                                stop=(ds == 1),
                            )
                        nc.vector.tensor_copy(qkT[:, ec, FH * h : FH * (h + 1)], ps)

                # ---- B: v [n-chunk, d] = cur-matmul(w_v) ----
                v_sb = v_pool.tile([P, 8, G_DIM], mdt)
                for nk in range(8):
                    ps = ps_mm.tile([P, FH], f32, tag="ps_mm")
                    for ds in range(2):
                        nc.tensor.matmul(
                            ps[:, :G_DIM],
                            (cur[:, ds, P * nk : P * (nk + 1)]),
                            (wq_sb[:, i, ds, 2 * G_DIM : 3 * G_DIM]),
                            start=(ds == 0),
                            stop=(ds == 1),
                        )
                    nc.vector.tensor_copy(v_sb[:, nk], ps[:, :G_DIM])

                # next group's x slice (overwritten into cur_next, then += hx)
                cur_next = None
                if i + 1 < G:
                    cur_next = cur_pool.tile([P, 2, N], mdt, tag="cur")
                    nc.sync.dma_start(
                        out=cur_next,
                        in_=xT[b, G_DIM * (i + 1) : G_DIM * (i + 2)].rearrange(
                            "(ds p) n -> p ds n", p=P
                        ),
                    )

                hx = hx_pool.tile([P, 2, N], mdt)

                # ---- attention, one q-half at a time ----
                for h in range(2):
                    pv_ps = [
                        ps_pv.tile([P, FH], f32, tag="ps_pv", name=f"pv_ps{dc}")
                        for dc in range(2)
                    ]
                    den_ps = ps_den.tile([1, FH], f32, tag="ps_den")
                    for kc in range(8):
                        s_ps = ps_s.tile([P, FH], f32, tag="ps_s")
                        for ds in range(2):
                            nc.tensor.matmul(
                                s_ps,
                                (qkT[:, 2 + ds, P * kc : P * (kc + 1)]),
                                (qkT[:, ds, FH * h : FH * (h + 1)]),
                                start=(ds == 0),
                                stop=(ds == 1),
                            )
                        pt = pt_pool.tile([P, FH], mdt)
                        nc.scalar.activation(pt, s_ps, Exp, scale=SCALE)
                        nc.tensor.matmul(
                            den_ps,
                            (ones_col),
                            (pt),
                            start=(kc == 0),
                            stop=(kc == 7),
                        )
                        for dc in range(2):
                            nc.tensor.matmul(
                                pv_ps[dc],
                                (v_sb[:, kc, P * dc : P * (dc + 1)]),
                                (pt),
                                start=(kc == 0),
                                stop=(kc == 7),
                            )

                    # denominators -> reciprocal on row 0, broadcast across
                    # partitions with an all-ones matmul (rows 1.. are zero)
                    nc.vector.reciprocal(rec_rows[h], den_ps)
                    with nc.allow_low_precision(reason="fp32r bcast staging"):
                        nc.vector.tensor_copy(rec_pads[h][0:1, :], rec_rows[h])
                    bc_ps = ps_mm.tile([P, FH], f32, tag="ps_mm")
                    nc.tensor.matmul(
                        bc_ps, (ones_sq), (rec_pads[h]),
                        start=True, stop=True,
                    )
                    rec_b = sm_pool.tile([P, FH], f32, tag="rec_b")
                    nc.vector.tensor_copy(rec_b, bc_ps)

                    for dc in range(2):
                        nc.vector.tensor_mul(
                            hx[:, dc, FH * h : FH * (h + 1)], pv_ps[dc], rec_b
                        )
                        if cur_next is not None:
                            nc.vector.tensor_add(
                                cur_next[:, dc, FH * h : FH * (h + 1)],
                                cur_next[:, dc, FH * h : FH * (h + 1)],
                                hx[:, dc, FH * h : FH * (h + 1)],
                            )

                # ---- projection partial of the PREVIOUS group (fills PE
                # during this group's softmax tail) ----
                for hx_done, gi, pw_done in ([(prev_hx[0], prev_hx[1], prev_hx[2])] if prev_hx else []):
                    _emit_proj_partial(nc, ps_mm, out_acc, hx_done, gi, pw_done, pb_sb, f32, Ident)
                prev_hx = (hx, i, pw_s)
                cur = cur_next

            # last group's projection partial, then store out^T
            _emit_proj_partial(nc, ps_mm, out_acc, prev_hx[0], prev_hx[1], prev_hx[2], pb_sb, f32, Ident)
            for ec in range(DIM // P):
                nc.sync.dma_start(
                    out=outT[b, P * ec : P * (ec + 1)], in_=out_acc[:, ec]
                )

    nc.finalize()
    return nc


def _emit_proj_partial(nc, ps_mm, out_acc, hx, gi, pw_s, pb_sb, f32, Ident):
    """out_acc[:, ec, :] (+)= pw_s.T-matmul(hx);  group 0 also adds bias."""
    for ec in range(DIM // P):
        for h in range(2):
            ps = ps_mm.tile([P, FH], f32, tag="ps_mm")
            for ds in range(2):
                nc.tensor.matmul(
                    ps,
                    (pw_s[:, ds, P * ec : P * (ec + 1)]),
                    (hx[:, ds, FH * h : FH * (h + 1)]),
                    start=(ds == 0),
                    stop=(ds == 1),
                )
            dst = out_acc[:, ec, FH * h : FH * (h + 1)]
            if gi == 0:
                nc.scalar.activation(dst, ps, Ident, bias=pb_sb[:, ec : ec + 1])
            else:
                nc.vector.tensor_add(dst, dst, ps)


def _host_prep(x, qkv_w, proj_w, proj_b):
    xT = np.ascontiguousarray(x.transpose(0, 2, 1))              # [B, DIM, N]
    wqkvT = np.ascontiguousarray(qkv_w.transpose(0, 2, 1))       # [G, 256, 768]
    pwT = np.ascontiguousarray(proj_w.T)                         # [DIM, DIM]
    pb = np.ascontiguousarray(proj_b.reshape(DIM // P, P).T)     # [128, 8]
    return xT, wqkvT, pwT, pb


def kernel(x, qkv_w, proj_w, proj_b):
    from concourse.bass_utils import run_bass_kernel_spmd

    x = np.asarray(x, dtype=np.float32)
    qkv_w = np.asarray(qkv_w, dtype=np.float32)
    proj_w = np.asarray(proj_w, dtype=np.float32)
    proj_b = np.asarray(proj_b, dtype=np.float32)

    xT, wqkvT, pwT, pb = _host_prep(x, qkv_w, proj_w, proj_b)

    nc = build_nc()
    in_maps = [
        {
            "xT": np.ascontiguousarray(xT[c * B_PER : (c + 1) * B_PER]),
            "wqkvT": wqkvT,
            "pwT": pwT,
            "pb": pb,
        }
        for c in range(N_CORES)
    ]
    res = run_bass_kernel_spmd(nc, in_maps, core_ids=list(range(N_CORES)))
    shards = [res.results[c]["outT"] for c in range(N_CORES)]
    outT = np.concatenate(shards, axis=0)          # [B, DIM, N]
    return np.ascontiguousarray(outT.transpose(0, 2, 1)).astype(np.float32)


if __name__ == "__main__":
    import sys

    if len(sys.argv) > 1 and sys.argv[1] == "build":
        nc = build_nc()
        print("build OK, instructions:", sum(1 for _ in nc.m.functions[0].instructions)
              if hasattr(nc.m.functions[0], "instructions") else "?")



# revision 2
# speedup vs baseline: 2.1655x; 2.1655x over previous
"""HeadFusionAttention Trainium2 kernel v2 (8 NeuronCores, data-parallel over B).

Structure (per core: 2 batches, interleaved group-by-group):
  - All matmul operands in bf16 (PE runs bf16 at the same cycles/matmul as
    fp32r but with FWL weight loads; halves SBUF and input DMA traffic;
    rel-err budget 2e-2 >> bf16's ~3e-3).
  - The two batches advance through the 4 sequential groups in lockstep
    (b0.g0, b1.g0, b0.g1, ...): during one batch's softmax-normalize /
    head-fusion-add serial chain (DVE), PE runs the other batch's matmuls.
  - Projection deferred to batch end: every (ec, h) output tile accumulates
    its 8 d-chunk matmuls in a single PSUM chain (no cross-group DVE adds),
    emitted group-3-chunk-first so a chain never parks on its PSUM bank.
  - Softmax denominator via ones[128,128] matmul -> broadcast denominator
    directly; exp has no max-subtraction (scores are O(1) by construction).
  - qkT PSUM evacuations on ACT (ScalarE), v on DVE: at group boundaries DVE
    runs the normalize chain and would otherwise starve ps_mm.
  - kc loop software-pipelined: S(kc+1) issues while ACT exp(kc) runs.
  - build_nc(reps=N) wraps the body in tc.For_i for differential device-time
    measurement (see test.py).
"""

import numpy as np

B, N, DIM = 16, 1024, 1024
G = 4
G_DIM = 256
SCALE = 128 ** -0.5
N_CORES = 8
B_PER = B // N_CORES  # 2

P = 128          # SBUF partitions
FH = 512         # free-dim half (psum bank: 512 fp32)


def build_nc(reps=None):
    from contextlib import ExitStack

    import concourse.mybir as mybir
    import concourse.tile as tile
    from concourse import bacc

    f32 = mybir.dt.float32
    mdt = mybir.dt.bfloat16

    nc = bacc.Bacc()

    xT = nc.declare_dram_parameter("xT", [B_PER, DIM, N], mdt, isOutput=False)
    wqkvT = nc.declare_dram_parameter("wqkvT", [G, G_DIM, 3 * G_DIM], mdt, isOutput=False)
    pwT = nc.declare_dram_parameter("pwT", [DIM, DIM], mdt, isOutput=False)
    pb = nc.declare_dram_parameter("pb", [P, DIM // P], f32, isOutput=False)
    outT = nc.declare_dram_parameter("outT", [B_PER, DIM, N], f32, isOutput=True)

    Exp = mybir.ActivationFunctionType.Exp
    Ident = mybir.ActivationFunctionType.Identity

    with tile.TileContext(nc) as tc, ExitStack() as ctx:
        consts = ctx.enter_context(tc.tile_pool(name="consts", bufs=1))
        cur_pool = ctx.enter_context(tc.tile_pool(name="cur_pool", bufs=5))
        qk_pool = ctx.enter_context(tc.tile_pool(name="qk_pool", bufs=3))
        v_pool = ctx.enter_context(tc.tile_pool(name="v_pool", bufs=3))
        pt_pool = ctx.enter_context(tc.tile_pool(name="pt_pool", bufs=6))
        hx_pool = ctx.enter_context(tc.tile_pool(name="hx_pool", bufs=9))
        sm_pool = ctx.enter_context(tc.tile_pool(name="sm_pool", bufs=3))
        dn_pool = ctx.enter_context(tc.tile_pool(name="dn_pool", bufs=8))
        out_pool = ctx.enter_context(tc.tile_pool(name="out_pool", bufs=4))

        ps_mm = ctx.enter_context(tc.tile_pool(name="ps_mm", bufs=2, space="PSUM"))
        ps_s = ctx.enter_context(tc.tile_pool(name="ps_s", bufs=2, space="PSUM"))
        ps_pv = ctx.enter_context(tc.tile_pool(name="ps_pv", bufs=2, space="PSUM"))
        ps_den = ctx.enter_context(tc.tile_pool(name="ps_den", bufs=1, space="PSUM"))
        ps_proj = ctx.enter_context(tc.tile_pool(name="ps_proj", bufs=1, space="PSUM"))

        # ---- constants (loaded once, outside any rep loop) ----
        # DMA order matters: first compute needs cur(b0) + wq[g0] only; the
        # 2MB pwT (needed ~40us in) goes last.
        wq_sb = consts.tile([P, G, 2, 3 * G_DIM], mdt)
        pb_sb = consts.tile([P, DIM // P], f32)
        pw_sb = consts.tile([P, DIM // P, DIM], mdt)

        def load_weights():
            nc.sync.dma_start(
                out=wq_sb[:, 0],
                in_=wqkvT[0].rearrange("(ds p) e -> p ds e", p=P),
            )
            nc.sync.dma_start(out=pb_sb, in_=pb[:, :])
            for g in range(1, G):
                nc.sync.dma_start(
                    out=wq_sb[:, g],
                    in_=wqkvT[g].rearrange("(ds p) e -> p ds e", p=P),
                )
            nc.sync.dma_start(
                out=pw_sb, in_=pwT.rearrange("(dc p) e -> p dc e", p=P)
            )

        ones_f32 = consts.tile([P, P], f32)
        nc.vector.memset(ones_f32, 1.0)
        ones_sq = consts.tile([P, P], mdt)
        nc.vector.tensor_copy(ones_sq, ones_f32)

        def emit_qkv(cur, i):
            """q^T,k^T [e-chunk, n] and v [n-chunk, d] for group i."""
            qkT = qk_pool.tile([P, 4, N], mdt)
            for h in range(2):
                for ec in range(4):
                    ps = ps_mm.tile([P, FH], f32, tag="ps_mm")
                    for ds in range(2):
                        nc.tensor.matmul(
                            ps,
                            (wq_sb[:, i, ds, P * ec : P * (ec + 1)]),
                            (cur[:, ds, FH * h : FH * (h + 1)]),
                            start=(ds == 0),
                            stop=(ds == 1),
                        )
                    nc.scalar.activation(qkT[:, ec, FH * h : FH * (h + 1)], ps, Ident)
            v_sb = v_pool.tile([P, 8, G_DIM], mdt)
            for nk in range(8):
                ps = ps_mm.tile([P, FH], f32, tag="ps_mm")
                for ds in range(2):
                    nc.tensor.matmul(
                        ps[:, :G_DIM],
                        (cur[:, ds, P * nk : P * (nk + 1)]),
                        (wq_sb[:, i, ds, 2 * G_DIM : 3 * G_DIM]),
                        start=(ds == 0),
                        stop=(ds == 1),
                    )
                nc.vector.tensor_copy(v_sb[:, nk], ps[:, :G_DIM])
            return qkT, v_sb

        def emit_attention(qkT, v_sb, cur_next):
            """softmax(q k^T) v, normalized; accumulates into cur_next."""
            hx = hx_pool.tile([P, 2, N], mdt)
            for h in range(2):
                pv_ps = [
                    ps_pv.tile([P, FH], f32, tag="ps_pv", name=f"pv_ps{dc}")
                    for dc in range(2)
                ]

                def emit_s(kc):
                    s_ps = ps_s.tile([P, FH], f32, tag="ps_s")
                    for ds in range(2):
                        nc.tensor.matmul(
                            s_ps,
                            (qkT[:, 2 + ds, P * kc : P * (kc + 1)]),
                            (qkT[:, ds, FH * h : FH * (h + 1)]),
                            start=(ds == 0),
                            stop=(ds == 1),
                        )
                    pt = pt_pool.tile([P, FH], mdt)
                    nc.scalar.activation(pt, s_ps, Exp, scale=SCALE)
                    return pt

                def emit_pv(kc, pt):
                    for dc in range(2):
                        nc.tensor.matmul(
                            pv_ps[dc],
                            (v_sb[:, kc, P * dc : P * (dc + 1)]),
                            (pt),
                            start=(kc == 0),
                            stop=(kc == 7),
                        )

                # software-pipelined: S(kc)+exp(kc) one step ahead of pv;
                # pt tiles also tree-summed on DVE (f32) so the partition
                # reduction needs a single ones-matmul instead of 8
                prev = None
                pts, l1 = [], []
                for kc in range(8):
                    pt = emit_s(kc)
                    if prev is not None:
                        emit_pv(kc - 1, prev)
                    prev = pt
                    pts.append(pt)
                    if kc % 2 == 1:
                        s = dn_pool.tile([P, FH], mdt, tag="dn", name="dn_l1")
                        nc.vector.tensor_add(s, pts[kc - 1], pts[kc])
                        l1.append(s)
                emit_pv(7, prev)
                l2a = dn_pool.tile([P, FH], mdt, tag="dn", name="dn_l2a")
                nc.vector.tensor_add(l2a, l1[0], l1[1])
                l2b = dn_pool.tile([P, FH], mdt, tag="dn", name="dn_l2b")
                nc.vector.tensor_add(l2b, l1[2], l1[3])
                root = dn_pool.tile([P, FH], mdt, tag="dnroot", name="dn_root")
                nc.vector.tensor_add(root, l2a, l2b)
                den_ps = ps_den.tile([P, FH], f32, tag="ps_den")
                # ones_sq weight -> every den_ps row is the denominator
                # (broadcast built into the partition reduction)
                nc.tensor.matmul(den_ps, (ones_sq), (root), start=True, stop=True)

                rec_b = sm_pool.tile([P, FH], f32, tag="rec_b")
                nc.vector.reciprocal(rec_b, den_ps)
                for dc in range(2):
                    nc.vector.tensor_mul(
                        hx[:, dc, FH * h : FH * (h + 1)], pv_ps[dc], rec_b
                    )
                    if cur_next is not None:
                        nc.vector.tensor_add(
                            cur_next[:, dc, FH * h : FH * (h + 1)],
                            cur_next[:, dc, FH * h : FH * (h + 1)],
                            hx[:, dc, FH * h : FH * (h + 1)],
                        )
            return hx

        def emit_proj(hxs, b):
            """out[ec,h] = sum_dc pw.T @ hx (+bias), one PSUM chain each."""
            for ec in range(DIM // P):
                for h in range(2):
                    ps = ps_proj.tile([P, FH], f32, tag="ps_proj")
                    # group-3 chunks first: the chain's first dep is the last
                    # hx, so a chain never parks on its PSUM bank waiting
                    for jj, j in enumerate([6, 7, 0, 1, 2, 3, 4, 5]):
                        g, ds = j // 2, j % 2
                        nc.tensor.matmul(
                            ps,
                            (pw_sb[:, j, P * ec : P * (ec + 1)]),
                            (hxs[g][:, ds, FH * h : FH * (h + 1)]),
                            start=(jj == 0),
                            stop=(jj == 7),
                        )
                    ot = out_pool.tile([P, FH], f32, tag="ot")
                    nc.scalar.activation(ot, ps, Ident, bias=pb_sb[:, ec : ec + 1])
                    nc.sync.dma_start(
                        out=outT[b, P * ec : P * (ec + 1), FH * h : FH * (h + 1)],
                        in_=ot,
                    )

        def body(first=False):
            cur = [None] * B_PER
            hxs = [[] for _ in range(B_PER)]
            for b in range(B_PER):
                cur[b] = cur_pool.tile([P, 2, N], mdt, tag="cur", name="cur")
                nc.sync.dma_start(
                    out=cur[b],
                    in_=xT[b, 0:G_DIM].rearrange("(ds p) n -> p ds n", p=P),
                )
                if first and b == 0:
                    load_weights()

            # the two batches advance in lockstep: each batch's serial
            # normalize chain overlaps the other batch's matmuls
            for i in range(G):
                for b in range(B_PER):
                    qkT, v_sb = emit_qkv(cur[b], i)
                    cur_next = None
                    if i + 1 < G:
                        cur_next = cur_pool.tile([P, 2, N], mdt, tag="cur")
                        nc.sync.dma_start(
                            out=cur_next,
                            in_=xT[b, G_DIM * (i + 1) : G_DIM * (i + 2)].rearrange(
                                "(ds p) n -> p ds n", p=P
                            ),
                        )
                    hxs[b].append(emit_attention(qkT, v_sb, cur_next))
                    cur[b] = cur_next

            for b in range(B_PER):
                emit_proj(hxs[b], b)

        if reps is None:
            body(first=True)
        else:
            load_weights()
            with tc.For_i(0, reps, 1):
                body()

    nc.finalize()
    return nc


def _host_prep(x, qkv_w, proj_w, proj_b):
    import ml_dtypes

    bf16 = ml_dtypes.bfloat16
    xT = np.ascontiguousarray(x.transpose(0, 2, 1)).astype(bf16)        # [B, DIM, N]
    wqkvT = np.ascontiguousarray(qkv_w.transpose(0, 2, 1)).astype(bf16)  # [G, 256, 768]
    pwT = np.ascontiguousarray(proj_w.T).astype(bf16)                    # [DIM, DIM]
    pb = np.ascontiguousarray(proj_b.reshape(DIM // P, P).T)             # [128, 8] f32
    return xT, wqkvT, pwT, pb


def kernel(x, qkv_w, proj_w, proj_b):
    from concourse.bass_utils import run_bass_kernel_spmd

    x = np.asarray(x, dtype=np.float32)
    qkv_w = np.asarray(qkv_w, dtype=np.float32)
    proj_w = np.asarray(proj_w, dtype=np.float32)
    proj_b = np.asarray(proj_b, dtype=np.float32)

    xT, wqkvT, pwT, pb = _host_prep(x, qkv_w, proj_w, proj_b)

    nc = build_nc()
    in_maps = [
        {
            "xT": np.ascontiguousarray(xT[c * B_PER : (c + 1) * B_PER]),
            "wqkvT": wqkvT,
            "pwT": pwT,
            "pb": pb,
        }
        for c in range(N_CORES)
    ]
    res = run_bass_kernel_spmd(nc, in_maps, core_ids=list(range(N_CORES)))
    shards = [res.results[c]["outT"] for c in range(N_CORES)]
    outT = np.concatenate(shards, axis=0)          # [B, DIM, N]
    return np.ascontiguousarray(outT.transpose(0, 2, 1)).astype(np.float32)


# revision 6
# speedup vs baseline: 19.4365x; 8.9756x over previous
"""HeadFusionAttention Trainium2 kernel (8 NeuronCores, data-parallel over B).

Per core: 2 batches advancing through the 4 sequential head-fusion groups in
lockstep (b0.g0, b1.g0, b0.g1, ...) so one batch's serial softmax-normalize
chain (DVE) always overlaps the other batch's matmuls on PE.

  - Matmul operands in bf16 (same PE cycles as fp32r, FWL weight loads, half
    the SBUF/DMA), except q/k which evacuate as fp8e4m3: the q.k^T score
    matmul runs in DoubleRow perf mode, contracting both 128-row d-chunks in
    one pass (half the matmuls). Measured rel err 9.0e-3 vs the 2e-2 gate.
  - x resident in SBUF (one load per batch); the head-fusion add writes
    cur_next = x_slice + hx directly, no mid-kernel load dependencies.
  - Projection deferred to batch end: each (ec, h) output tile accumulates
    all 8 d-chunk matmuls in one PSUM chain (no cross-group DVE adds),
    emitted group-3-chunk-first so a chain never parks on its PSUM bank.
    Output tiles collect in SBUF and leave as one bf16 DMA per (batch, half)
    (many small output DMAs inside a For_i rep loop crash the device).
  - Softmax denominator: pt tiles tree-summed on DVE (bf16), then a single
    ones[128,128] matmul produces the partition-reduced denominator already
    broadcast across partitions; exp needs no max-subtraction (scores O(1)).
  - kc loop software-pipelined 2 deep (PE runs S(kc+1), S(kc+2) while ACT's
    exp(kc) is in flight); PSUM: qkv+S share 4 banks, pv 2, den+proj 2.
  - PSUM evacuations split across ACT and DVE so neither gates PE.
  - build_nc(reps=N) wraps the body in tc.For_i (branch-hinted back edge,
    rep-counter witness output) for differential device timing (see test.py).
"""

import numpy as np

B, N, DIM = 16, 1024, 1024
G = 4
G_DIM = 256
SCALE = 128 ** -0.5
N_CORES = 8
B_PER = B // N_CORES  # 2

P = 128          # SBUF partitions
FH = 512         # free-dim half (psum bank: 512 fp32)
FP8_S = True     # q/k in fp8e4m3 + DoubleRow S matmuls (halves S's PE time;
                 # measured rel err ~9e-3 vs the 2e-2 gate)


def build_nc(reps=None):
    from contextlib import ExitStack

    import concourse.mybir as mybir
    import concourse.tile as tile
    from concourse import bacc

    f32 = mybir.dt.float32
    mdt = mybir.dt.bfloat16
    qdt = mybir.dt.float8e4 if FP8_S else mdt
    DR = mybir.MatmulPerfMode.DoubleRow

    nc = bacc.Bacc()

    xT = nc.declare_dram_parameter("xT", [B_PER, DIM, N], mdt, isOutput=False)
    wqkvT = nc.declare_dram_parameter("wqkvT", [G, G_DIM, 3 * G_DIM], mdt, isOutput=False)
    pwT = nc.declare_dram_parameter("pwT", [DIM, DIM], mdt, isOutput=False)
    pb = nc.declare_dram_parameter("pb", [P, DIM // P], f32, isOutput=False)
    outT = nc.declare_dram_parameter("outT", [B_PER, DIM, N], mdt, isOutput=True)

    Exp = mybir.ActivationFunctionType.Exp
    Ident = mybir.ActivationFunctionType.Identity

    with tile.TileContext(nc) as tc, ExitStack() as ctx:
        consts = ctx.enter_context(tc.tile_pool(name="consts", bufs=1))
        x_pool = ctx.enter_context(tc.tile_pool(name="x_pool", bufs=2))
        cur_pool = ctx.enter_context(tc.tile_pool(name="cur_pool", bufs=4))
        qk_pool = ctx.enter_context(tc.tile_pool(name="qk_pool", bufs=3))
        v_pool = ctx.enter_context(tc.tile_pool(name="v_pool", bufs=3))
        pt_pool = ctx.enter_context(tc.tile_pool(name="pt_pool", bufs=6))
        hx_pool = ctx.enter_context(tc.tile_pool(name="hx_pool", bufs=9))
        sm_pool = ctx.enter_context(tc.tile_pool(name="sm_pool", bufs=3))
        dn_pool = ctx.enter_context(tc.tile_pool(name="dn_pool", bufs=8))
        out_pool = ctx.enter_context(tc.tile_pool(name="out_pool", bufs=3))

        # 8 PSUM banks total: qkv+S share 4 (both cycle fast), pv holds 2
        # accumulators, den (brief, tree-root only) + proj chains share 2
        ps_a = ctx.enter_context(tc.tile_pool(name="ps_a", bufs=4, space="PSUM"))
        ps_pv = ctx.enter_context(tc.tile_pool(name="ps_pv", bufs=2, space="PSUM"))
        ps_dp = ctx.enter_context(tc.tile_pool(name="ps_dp", bufs=2, space="PSUM"))

        # ---- constants (loaded once, outside any rep loop) ----
        # DMA order matters: first compute needs cur(b0) + wq[g0] only; the
        # 2MB pwT (needed ~40us in) goes last.
        wq_sb = consts.tile([P, G, 2, 3 * G_DIM], mdt)
        pb_sb = consts.tile([P, DIM // P], f32)
        pw_sb = consts.tile([P, DIM // P, DIM], mdt)

        def load_weights():
            nc.sync.dma_start(
                out=wq_sb[:, 0],
                in_=wqkvT[0].rearrange("(ds p) e -> p ds e", p=P),
            )
            nc.sync.dma_start(out=pb_sb, in_=pb[:, :])
            for g in range(1, G):
                nc.sync.dma_start(
                    out=wq_sb[:, g],
                    in_=wqkvT[g].rearrange("(ds p) e -> p ds e", p=P),
                )
            nc.sync.dma_start(
                out=pw_sb, in_=pwT.rearrange("(dc p) e -> p dc e", p=P)
            )

        ones_f32 = consts.tile([P, P], f32)
        nc.vector.memset(ones_f32, 1.0)
        ones_sq = consts.tile([P, P], mdt)
        nc.vector.tensor_copy(ones_sq, ones_f32)

        def emit_qkv(cur, i):
            """q^T,k^T [e-chunk, n] and v [n-chunk, d] for group i.

            With FP8_S the q/k halves evacuate as fp8e4m3 laid out
            [p, qk, ds, n] so the S matmul's DoubleRow APs ([K, 2(ds), M])
            slice directly.
            """
            qkT = qk_pool.tile([P, 2, 2, N], qdt)
            for h in range(2):
                for ec in range(4):
                    ps = ps_a.tile([P, FH], f32, tag="ps_a")
                    for ds in range(2):
                        nc.tensor.matmul(
                            ps,
                            (wq_sb[:, i, ds, P * ec : P * (ec + 1)]),
                            (cur[:, ds, FH * h : FH * (h + 1)]),
                            start=(ds == 0),
                            stop=(ds == 1),
                        )
                    # alternate ACT/DVE so neither engine gates the PE
                    dst = qkT[:, ec // 2, ec % 2, FH * h : FH * (h + 1)]
                    if (ec + h) % 2 == 0:
                        nc.scalar.activation(dst, ps, Ident)
                    else:
                        nc.vector.tensor_copy(dst, ps)
            v_sb = v_pool.tile([P, 8, G_DIM], mdt)
            for nk in range(8):
                ps = ps_a.tile([P, FH], f32, tag="ps_a")
                for ds in range(2):
                    nc.tensor.matmul(
                        ps[:, :G_DIM],
                        (cur[:, ds, P * nk : P * (nk + 1)]),
                        (wq_sb[:, i, ds, 2 * G_DIM : 3 * G_DIM]),
                        start=(ds == 0),
                        stop=(ds == 1),
                    )
                nc.vector.tensor_copy(v_sb[:, nk], ps[:, :G_DIM])
            return qkT, v_sb

        def emit_attention(qkT, v_sb, cur_next, x_next):
            """softmax(q k^T) v, normalized; cur_next = x_next + hx."""
            hx = hx_pool.tile([P, 2, N], mdt)
            for h in range(2):
                pv_ps = [
                    ps_pv.tile([P, FH], f32, tag="ps_pv", name=f"pv_ps{dc}")
                    for dc in range(2)
                ]

                def emit_s(kc):
                    s_ps = ps_a.tile([P, FH], f32, tag="ps_a", name="s_ps")
                    if FP8_S:
                        # DoubleRow: one matmul contracts both 128-row ds
                        # chunks (virtual 128x256 array)
                        nc.tensor.matmul(
                            s_ps,
                            (qkT[:, 1, :, P * kc : P * (kc + 1)]),
                            (qkT[:, 0, :, FH * h : FH * (h + 1)]),
                            start=True,
                            stop=True,
                            perf_mode=DR,
                        )
                    else:
                        for ds in range(2):
                            nc.tensor.matmul(
                                s_ps,
                                (qkT[:, 1, ds, P * kc : P * (kc + 1)]),
                                (qkT[:, 0, ds, FH * h : FH * (h + 1)]),
                                start=(ds == 0),
                                stop=(ds == 1),
                            )
                    pt = pt_pool.tile([P, FH], mdt)
                    nc.scalar.activation(pt, s_ps, Exp, scale=SCALE)
                    return pt

                def emit_pv(kc, pt):
                    for dc in range(2):
                        nc.tensor.matmul(
                            pv_ps[dc],
                            (v_sb[:, kc, P * dc : P * (dc + 1)]),
                            (pt),
                            start=(kc == 0),
                            stop=(kc == 7),
                        )

                # software-pipelined: S(kc)+exp(kc) one step ahead of pv;
                # pt tiles also tree-summed on DVE (f32) so the partition
                # reduction needs a single ones-matmul instead of 8
                pts, l1 = [], []
                for kc in range(8):
                    pts.append(emit_s(kc))
                    if kc >= 2:
                        emit_pv(kc - 2, pts[kc - 2])
                    if kc % 2 == 1:
                        s = dn_pool.tile([P, FH], mdt, tag="dn", name="dn_l1")
                        nc.vector.tensor_add(s, pts[kc - 1], pts[kc])
                        l1.append(s)
                emit_pv(6, pts[6])
                emit_pv(7, pts[7])
                l2a = dn_pool.tile([P, FH], mdt, tag="dn", name="dn_l2a")
                nc.vector.tensor_add(l2a, l1[0], l1[1])
                l2b = dn_pool.tile([P, FH], mdt, tag="dn", name="dn_l2b")
                nc.vector.tensor_add(l2b, l1[2], l1[3])
                root = dn_pool.tile([P, FH], mdt, tag="dnroot", name="dn_root")
                nc.vector.tensor_add(root, l2a, l2b)
                den_ps = ps_dp.tile([P, FH], f32, tag="ps_dp", name="den_ps")
                # ones_sq weight -> every den_ps row is the denominator
                # (broadcast built into the partition reduction)
                nc.tensor.matmul(den_ps, (ones_sq), (root), start=True, stop=True)

                rec_b = sm_pool.tile([P, FH], f32, tag="rec_b")
                nc.vector.reciprocal(rec_b, den_ps)
                for dc in range(2):
                    nc.vector.tensor_mul(
                        hx[:, dc, FH * h : FH * (h + 1)], pv_ps[dc], rec_b
                    )
                    if cur_next is not None:
                        nc.vector.tensor_add(
                            cur_next[:, dc, FH * h : FH * (h + 1)],
                            x_next[:, dc, FH * h : FH * (h + 1)],
                            hx[:, dc, FH * h : FH * (h + 1)],
                        )
            return hx

        def emit_proj(hxs, b):
            """out[ec,h] = sum_dc pw.T @ hx (+bias), one PSUM chain each.

            Output tiles accumulate in SBUF and leave as one DMA per (b, h):
            many small per-tile output DMAs inside a For_i rep loop crash the
            device (NRT_EXEC_UNIT_UNRECOVERABLE), and fewer/bigger transfers
            are cheaper anyway.
            """
            for h in range(2):
                oa = out_pool.tile([P, DIM // P, FH], mdt, tag="oa", name="oa")
                for ec in range(DIM // P):
                    ps = ps_dp.tile([P, FH], f32, tag="ps_dp", name="proj_ps")
                    # group-3 chunks first: the chain's first dep is the last
                    # hx, so a chain never parks on its PSUM bank waiting
                    for jj, j in enumerate([6, 7, 0, 1, 2, 3, 4, 5]):
                        g, ds = j // 2, j % 2
                        nc.tensor.matmul(
                            ps,
                            (pw_sb[:, j, P * ec : P * (ec + 1)]),
                            (hxs[g][:, ds, FH * h : FH * (h + 1)]),
                            start=(jj == 0),
                            stop=(jj == 7),
                        )
                    # DVE, not ACT: ACT is the busier evacuation engine here
                    nc.vector.tensor_scalar_add(oa[:, ec], ps, pb_sb[:, ec : ec + 1])
                nc.sync.dma_start(
                    out=outT[b, :, FH * h : FH * (h + 1)].rearrange(
                        "(ec p) f -> p ec f", p=P
                    ),
                    in_=oa,
                )

        def body(first=False):
            # x resident in SBUF: one small DMA (group 0) + one big DMA
            # (groups 1-3) per batch; the head-fusion add then reads the
            # resident slice directly, so no mid-kernel load dependencies
            x_sb = [None] * B_PER
            hxs = [[] for _ in range(B_PER)]
            for b in range(B_PER):
                x_sb[b] = x_pool.tile([P, 2 * G, N], mdt, tag="x_sb", name="x_sb")
                xr = xT[b].rearrange("(c p) n -> p c n", p=P)
                nc.sync.dma_start(out=x_sb[b][:, 0:2], in_=xr[:, 0:2])
                if first and b == 0:
                    load_weights()
                nc.sync.dma_start(out=x_sb[b][:, 2:], in_=xr[:, 2:])

            # the two batches advance in lockstep: each batch's serial
            # normalize chain overlaps the other batch's matmuls
            cur = [x_sb[b][:, 0:2] for b in range(B_PER)]
            for i in range(G):
                for b in range(B_PER):
                    qkT, v_sb = emit_qkv(cur[b], i)
                    cur_next = None
                    if i + 1 < G:
                        cur_next = cur_pool.tile([P, 2, N], mdt, tag="cur", name="cur")
                    hxs[b].append(
                        emit_attention(qkT, v_sb, cur_next,
                                       None if cur_next is None
                                       else x_sb[b][:, 2 * (i + 1) : 2 * (i + 2)])
                    )
                    cur[b] = cur_next

            for b in range(B_PER):
                emit_proj(hxs[b], b)

        if reps is None:
            body(first=True)
        else:
            # rep-count witness: cnt output reads back exactly `reps` if
            # every iteration executed (guards the differential timing
            # against early-exit / corrupted loop state)
            cnt = nc.declare_dram_parameter("cnt", [1, 1], f32, isOutput=True)
            ct = consts.tile([1, 1], f32)
            nc.vector.memset(ct, 0.0)
            load_weights()
            from concourse.engine_type import EngineType
            # branch hints: the body far exceeds one IRAM block per busy
            # engine, so an unhinted back-edge pays a ~4us I$ fetch per rep
            with tc.For_i(0, reps, 1,
                          hint_engines=(EngineType.PE, EngineType.DVE,
                                        EngineType.Activation, EngineType.SP)):
                body()
                nc.vector.tensor_scalar_add(ct, ct, 1.0)
                nc.sync.dma_start(out=cnt[:, :], in_=ct)

    nc.finalize()
    return nc


def _host_prep(x, qkv_w, proj_w, proj_b):
    import ml_dtypes

    bf16 = ml_dtypes.bfloat16
    xT = np.ascontiguousarray(x.transpose(0, 2, 1)).astype(bf16)        # [B, DIM, N]
    wqkvT = np.ascontiguousarray(qkv_w.transpose(0, 2, 1)).astype(bf16)  # [G, 256, 768]
    pwT = np.ascontiguousarray(proj_w.T).astype(bf16)                    # [DIM, DIM]
    pb = np.ascontiguousarray(proj_b.reshape(DIM // P, P).T)             # [128, 8] f32
    return xT, wqkvT, pwT, pb


def kernel(x, qkv_w, proj_w, proj_b):
    from concourse.bass_utils import run_bass_kernel_spmd

    x = np.asarray(x, dtype=np.float32)
    qkv_w = np.asarray(qkv_w, dtype=np.float32)
    proj_w = np.asarray(proj_w, dtype=np.float32)
    proj_b = np.asarray(proj_b, dtype=np.float32)

    xT, wqkvT, pwT, pb = _host_prep(x, qkv_w, proj_w, proj_b)

    nc = build_nc()
    in_maps = [
        {
            "xT": np.ascontiguousarray(xT[c * B_PER : (c + 1) * B_PER]),
            "wqkvT": wqkvT,
            "pwT": pwT,
            "pb": pb,
        }
        for c in range(N_CORES)
    ]
    try:
        res = run_bass_kernel_spmd(nc, in_maps, core_ids=list(range(N_CORES)))
    except Exception:
        # the axon-tunneled device occasionally reports a transient
        # NRT_EXEC_UNIT_UNRECOVERABLE; a fresh run has always succeeded
        import time as _time

        _time.sleep(10)
        res = run_bass_kernel_spmd(nc, in_maps, core_ids=list(range(N_CORES)))
    shards = [res.results[c]["outT"] for c in range(N_CORES)]
    outT = np.concatenate(shards, axis=0)          # [B, DIM, N]
    return np.ascontiguousarray(outT.transpose(0, 2, 1)).astype(np.float32)


# revision 7
# speedup vs baseline: 22.0358x; 1.1337x over previous
"""HeadFusionAttention Trainium2 kernel (8 NeuronCores, data-parallel over B).

Per core: 2 batches advancing through the 4 sequential head-fusion groups in
lockstep (b0.g0, b1.g0, b0.g1, ...) so one batch's serial softmax-normalize
chain (DVE) always overlaps the other batch's matmuls on PE.

  - Matmul operands in bf16 (same PE cycles as fp32r, FWL weight loads, half
    the SBUF/DMA), except q/k which evacuate as fp8e4m3: the q.k^T score
    matmul runs in DoubleRow perf mode, contracting both 128-row d-chunks in
    one pass (half the matmuls). Measured rel err 9.0e-3 vs the 2e-2 gate.
  - x resident in SBUF (one load per batch); the head-fusion add writes
    cur_next = x_slice + hx directly, no mid-kernel load dependencies.
  - Projection deferred to batch end: each (ec, h) output tile accumulates
    all 8 d-chunk matmuls in one PSUM chain (no cross-group DVE adds),
    emitted group-3-chunk-first so a chain never parks on its PSUM bank.
    Output tiles collect in SBUF and leave as one bf16 DMA per (batch, half)
    (many small output DMAs inside a For_i rep loop crash the device).
  - Softmax denominator: pt tiles tree-summed on DVE (bf16), then a single
    ones[128,128] matmul produces the partition-reduced denominator already
    broadcast across partitions; exp needs no max-subtraction (scores O(1)).
  - kc loop software-pipelined 3 deep (PE runs up to S(kc+3) while ACT's
    exp(kc) is in flight); PSUM: qkv+S share 4 banks, pv 2, den+proj 2.
  - qkT evacuations on ACT, v (bank-paired, one [128,512] copy per two
    chains) and the normalize/den-tree work on DVE.
  - build_nc(reps=N) wraps the body in tc.For_i (branch-hinted back edge,
    rep-counter witness output) for differential device timing (see test.py).
"""

import numpy as np

B, N, DIM = 16, 1024, 1024
G = 4
G_DIM = 256
SCALE = 128 ** -0.5
N_CORES = 8
B_PER = B // N_CORES  # 2

P = 128          # SBUF partitions
FH = 512         # free-dim half (psum bank: 512 fp32)
FP8_S = True     # q/k in fp8e4m3 + DoubleRow S matmuls (halves S's PE time;
                 # measured rel err ~9e-3 vs the 2e-2 gate)


def build_nc(reps=None):
    from contextlib import ExitStack

    import concourse.mybir as mybir
    import concourse.tile as tile
    from concourse import bacc

    f32 = mybir.dt.float32
    mdt = mybir.dt.bfloat16
    qdt = mybir.dt.float8e4 if FP8_S else mdt
    DR = mybir.MatmulPerfMode.DoubleRow

    nc = bacc.Bacc()

    xT = nc.declare_dram_parameter("xT", [B_PER, DIM, N], mdt, isOutput=False)
    wqkvT = nc.declare_dram_parameter("wqkvT", [G, G_DIM, 3 * G_DIM], mdt, isOutput=False)
    pwT = nc.declare_dram_parameter("pwT", [DIM, DIM], mdt, isOutput=False)
    pb = nc.declare_dram_parameter("pb", [P, DIM // P], f32, isOutput=False)
    outT = nc.declare_dram_parameter("outT", [B_PER, DIM, N], mdt, isOutput=True)

    Exp = mybir.ActivationFunctionType.Exp
    Ident = mybir.ActivationFunctionType.Identity

    with tile.TileContext(nc) as tc, ExitStack() as ctx:
        consts = ctx.enter_context(tc.tile_pool(name="consts", bufs=1))
        x_pool = ctx.enter_context(tc.tile_pool(name="x_pool", bufs=2))
        cur_pool = ctx.enter_context(tc.tile_pool(name="cur_pool", bufs=4))
        qk_pool = ctx.enter_context(tc.tile_pool(name="qk_pool", bufs=4))
        v_pool = ctx.enter_context(tc.tile_pool(name="v_pool", bufs=4))
        pt_pool = ctx.enter_context(tc.tile_pool(name="pt_pool", bufs=9))
        hx_pool = ctx.enter_context(tc.tile_pool(name="hx_pool", bufs=9))
        sm_pool = ctx.enter_context(tc.tile_pool(name="sm_pool", bufs=4))
        dn_pool = ctx.enter_context(tc.tile_pool(name="dn_pool", bufs=10))
        out_pool = ctx.enter_context(tc.tile_pool(name="out_pool", bufs=3))

        # 8 PSUM banks total: qkv+S share 4 (both cycle fast), pv holds 2
        # accumulators, den (brief, tree-root only) + proj chains share 2
        ps_a = ctx.enter_context(tc.tile_pool(name="ps_a", bufs=4, space="PSUM"))
        ps_pv = ctx.enter_context(tc.tile_pool(name="ps_pv", bufs=2, space="PSUM"))
        ps_dp = ctx.enter_context(tc.tile_pool(name="ps_dp", bufs=2, space="PSUM"))

        # ---- constants (loaded once, outside any rep loop) ----
        # DMA order matters: first compute needs cur(b0) + wq[g0] only; the
        # 2MB pwT (needed ~40us in) goes last.
        wq_sb = consts.tile([P, G, 2, 3 * G_DIM], mdt)
        pb_sb = consts.tile([P, DIM // P], f32)
        pw_sb = consts.tile([P, DIM // P, DIM], mdt)

        def load_weights():
            nc.sync.dma_start(
                out=wq_sb[:, 0],
                in_=wqkvT[0].rearrange("(ds p) e -> p ds e", p=P),
            )
            nc.sync.dma_start(out=pb_sb, in_=pb[:, :])
            for g in range(1, G):
                nc.sync.dma_start(
                    out=wq_sb[:, g],
                    in_=wqkvT[g].rearrange("(ds p) e -> p ds e", p=P),
                )
            nc.sync.dma_start(
                out=pw_sb, in_=pwT.rearrange("(dc p) e -> p dc e", p=P)
            )

        ones_f32 = consts.tile([P, P], f32)
        nc.vector.memset(ones_f32, 1.0)
        ones_sq = consts.tile([P, P], mdt)
        nc.vector.tensor_copy(ones_sq, ones_f32)

        def emit_qkv(cur, i):
            """q^T,k^T [e-chunk, n] and v [n-chunk, d] for group i.

            With FP8_S the q/k halves evacuate as fp8e4m3 laid out
            [p, qk, ds, n] so the S matmul's DoubleRow APs ([K, 2(ds), M])
            slice directly.
            """
            qkT = qk_pool.tile([P, 2, 2, N], qdt)
            for h in range(2):
                for ec in range(4):
                    ps = ps_a.tile([P, FH], f32, tag="ps_a")
                    for ds in range(2):
                        nc.tensor.matmul(
                            ps,
                            (wq_sb[:, i, ds, P * ec : P * (ec + 1)]),
                            (cur[:, ds, FH * h : FH * (h + 1)]),
                            start=(ds == 0),
                            stop=(ds == 1),
                        )
                    nc.scalar.activation(
                        qkT[:, ec // 2, ec % 2, FH * h : FH * (h + 1)], ps, Ident
                    )
            v_sb = v_pool.tile([P, 8, G_DIM], mdt)
            for nk in range(0, 8, 2):
                ps = ps_a.tile([P, FH], f32, tag="ps_a")
                for half in range(2):
                    for ds in range(2):
                        nc.tensor.matmul(
                            ps[:, G_DIM * half : G_DIM * (half + 1)],
                            (cur[:, ds, P * (nk + half) : P * (nk + half + 1)]),
                            (wq_sb[:, i, ds, 2 * G_DIM : 3 * G_DIM]),
                            start=(ds == 0),
                            stop=(ds == 1),
                        )
                nc.vector.tensor_copy(v_sb[:, nk : nk + 2], ps)
            return qkT, v_sb

        def emit_attention(qkT, v_sb, cur_next, x_next):
            """softmax(q k^T) v, normalized; cur_next = x_next + hx."""
            hx = hx_pool.tile([P, 2, N], mdt)
            for h in range(2):
                pv_ps = [
                    ps_pv.tile([P, FH], f32, tag="ps_pv", name=f"pv_ps{dc}")
                    for dc in range(2)
                ]

                def emit_s(kc):
                    s_ps = ps_a.tile([P, FH], f32, tag="ps_a", name="s_ps")
                    if FP8_S:
                        # DoubleRow: one matmul contracts both 128-row ds
                        # chunks (virtual 128x256 array)
                        nc.tensor.matmul(
                            s_ps,
                            (qkT[:, 1, :, P * kc : P * (kc + 1)]),
                            (qkT[:, 0, :, FH * h : FH * (h + 1)]),
                            start=True,
                            stop=True,
                            perf_mode=DR,
                        )
                    else:
                        for ds in range(2):
                            nc.tensor.matmul(
                                s_ps,
                                (qkT[:, 1, ds, P * kc : P * (kc + 1)]),
                                (qkT[:, 0, ds, FH * h : FH * (h + 1)]),
                                start=(ds == 0),
                                stop=(ds == 1),
                            )
                    pt = pt_pool.tile([P, FH], mdt)
                    nc.scalar.activation(pt, s_ps, Exp, scale=SCALE)
                    return pt

                def emit_pv(kc, pt):
                    for dc in range(2):
                        nc.tensor.matmul(
                            pv_ps[dc],
                            (v_sb[:, kc, P * dc : P * (dc + 1)]),
                            (pt),
                            start=(kc == 0),
                            stop=(kc == 7),
                        )

                # software-pipelined: S(kc)+exp(kc) one step ahead of pv;
                # pt tiles also tree-summed on DVE (f32) so the partition
                # reduction needs a single ones-matmul instead of 8
                pts, l1 = [], []
                for kc in range(8):
                    pts.append(emit_s(kc))
                    if kc >= 3:
                        emit_pv(kc - 3, pts[kc - 3])
                    if kc % 2 == 1:
                        s = dn_pool.tile([P, FH], mdt, tag="dn", name="dn_l1")
                        nc.vector.tensor_add(s, pts[kc - 1], pts[kc])
                        l1.append(s)
                for kc in range(5, 8):
                    emit_pv(kc, pts[kc])
                l2a = dn_pool.tile([P, FH], mdt, tag="dn", name="dn_l2a")
                nc.vector.tensor_add(l2a, l1[0], l1[1])
                l2b = dn_pool.tile([P, FH], mdt, tag="dn", name="dn_l2b")
                nc.vector.tensor_add(l2b, l1[2], l1[3])
                root = dn_pool.tile([P, FH], mdt, tag="dnroot", name="dn_root")
                nc.vector.tensor_add(root, l2a, l2b)
                den_ps = ps_dp.tile([P, FH], f32, tag="ps_dp", name="den_ps")
                # ones_sq weight -> every den_ps row is the denominator
                # (broadcast built into the partition reduction)
                nc.tensor.matmul(den_ps, (ones_sq), (root), start=True, stop=True)

                rec_b = sm_pool.tile([P, FH], f32, tag="rec_b")
                nc.vector.reciprocal(rec_b, den_ps)
                for dc in range(2):
                    nc.vector.tensor_mul(
                        hx[:, dc, FH * h : FH * (h + 1)], pv_ps[dc], rec_b
                    )
                    if cur_next is not None:
                        nc.vector.tensor_add(
                            cur_next[:, dc, FH * h : FH * (h + 1)],
                            x_next[:, dc, FH * h : FH * (h + 1)],
                            hx[:, dc, FH * h : FH * (h + 1)],
                        )
            return hx

        def emit_proj(hxs, b):
            """out[ec,h] = sum_dc pw.T @ hx (+bias), one PSUM chain each.

            Output tiles accumulate in SBUF and leave as one DMA per (b, h):
            many small per-tile output DMAs inside a For_i rep loop crash the
            device (NRT_EXEC_UNIT_UNRECOVERABLE), and fewer/bigger transfers
            are cheaper anyway.
            """
            for h in range(2):
                oa = out_pool.tile([P, DIM // P, FH], mdt, tag="oa", name="oa")
                for ec in range(DIM // P):
                    ps = ps_dp.tile([P, FH], f32, tag="ps_dp", name="proj_ps")
                    # group-3 chunks first: the chain's first dep is the last
                    # hx, so a chain never parks on its PSUM bank waiting
                    for jj, j in enumerate([6, 7, 0, 1, 2, 3, 4, 5]):
                        g, ds = j // 2, j % 2
                        nc.tensor.matmul(
                            ps,
                            (pw_sb[:, j, P * ec : P * (ec + 1)]),
                            (hxs[g][:, ds, FH * h : FH * (h + 1)]),
                            start=(jj == 0),
                            stop=(jj == 7),
                        )
                    # DVE, not ACT: ACT is the busier evacuation engine here
                    nc.vector.tensor_scalar_add(oa[:, ec], ps, pb_sb[:, ec : ec + 1])
                nc.sync.dma_start(
                    out=outT[b, :, FH * h : FH * (h + 1)].rearrange(
                        "(ec p) f -> p ec f", p=P
                    ),
                    in_=oa,
                )

        def body(first=False):
            # x resident in SBUF: one small DMA (group 0) + one big DMA
            # (groups 1-3) per batch; the head-fusion add then reads the
            # resident slice directly, so no mid-kernel load dependencies
            x_sb = [None] * B_PER
            hxs = [[] for _ in range(B_PER)]
            for b in range(B_PER):
                x_sb[b] = x_pool.tile([P, 2 * G, N], mdt, tag="x_sb", name="x_sb")
                xr = xT[b].rearrange("(c p) n -> p c n", p=P)
                nc.sync.dma_start(out=x_sb[b][:, 0:2], in_=xr[:, 0:2])
                if first and b == 0:
                    load_weights()
                nc.sync.dma_start(out=x_sb[b][:, 2:], in_=xr[:, 2:])

            # the two batches advance in lockstep: each batch's serial
            # normalize chain overlaps the other batch's matmuls
            cur = [x_sb[b][:, 0:2] for b in range(B_PER)]
            for i in range(G):
                for b in range(B_PER):
                    qkT, v_sb = emit_qkv(cur[b], i)
                    cur_next = None
                    if i + 1 < G:
                        cur_next = cur_pool.tile([P, 2, N], mdt, tag="cur", name="cur")
                    hxs[b].append(
                        emit_attention(qkT, v_sb, cur_next,
                                       None if cur_next is None
                                       else x_sb[b][:, 2 * (i + 1) : 2 * (i + 2)])
                    )
                    cur[b] = cur_next

            for b in range(B_PER):
                emit_proj(hxs[b], b)

        if reps is None:
            body(first=True)
        else:
            # rep-count witness: cnt output reads back exactly `reps` if
            # every iteration executed (guards the differential timing
            # against early-exit / corrupted loop state)
            cnt = nc.declare_dram_parameter("cnt", [1, 1], f32, isOutput=True)
            ct = consts.tile([1, 1], f32)
            nc.vector.memset(ct, 0.0)
            load_weights()
            from concourse.engine_type import EngineType
            # branch hints: the body far exceeds one IRAM block per busy
            # engine, so an unhinted back-edge pays a ~4us I$ fetch per rep
            with tc.For_i(0, reps, 1,
                          hint_engines=(EngineType.PE, EngineType.DVE,
                                        EngineType.Activation, EngineType.SP)):
                body()
                nc.vector.tensor_scalar_add(ct, ct, 1.0)
                nc.sync.dma_start(out=cnt[:, :], in_=ct)

    nc.finalize()
    return nc


def _host_prep(x, qkv_w, proj_w, proj_b):
    import ml_dtypes

    bf16 = ml_dtypes.bfloat16
    xT = np.ascontiguousarray(x.transpose(0, 2, 1)).astype(bf16)        # [B, DIM, N]
    wqkvT = np.ascontiguousarray(qkv_w.transpose(0, 2, 1)).astype(bf16)  # [G, 256, 768]
    pwT = np.ascontiguousarray(proj_w.T).astype(bf16)                    # [DIM, DIM]
    pb = np.ascontiguousarray(proj_b.reshape(DIM // P, P).T)             # [128, 8] f32
    return xT, wqkvT, pwT, pb


def kernel(x, qkv_w, proj_w, proj_b):
    from concourse.bass_utils import run_bass_kernel_spmd

    x = np.asarray(x, dtype=np.float32)
    qkv_w = np.asarray(qkv_w, dtype=np.float32)
    proj_w = np.asarray(proj_w, dtype=np.float32)
    proj_b = np.asarray(proj_b, dtype=np.float32)

    xT, wqkvT, pwT, pb = _host_prep(x, qkv_w, proj_w, proj_b)

    nc = build_nc()
    in_maps = [
        {
            "xT": np.ascontiguousarray(xT[c * B_PER : (c + 1) * B_PER]),
            "wqkvT": wqkvT,
            "pwT": pwT,
            "pb": pb,
        }
        for c in range(N_CORES)
    ]
    try:
        res = run_bass_kernel_spmd(nc, in_maps, core_ids=list(range(N_CORES)))
    except Exception:
        # the axon-tunneled device occasionally reports a transient
        # NRT_EXEC_UNIT_UNRECOVERABLE; a fresh run has always succeeded
        import time as _time

        _time.sleep(10)
        res = run_bass_kernel_spmd(nc, in_maps, core_ids=list(range(N_CORES)))
    shards = [res.results[c]["outT"] for c in range(N_CORES)]
    outT = np.concatenate(shards, axis=0)          # [B, DIM, N]
    return np.ascontiguousarray(outT.transpose(0, 2, 1)).astype(np.float32)
